# revision 6
# baseline (speedup 1.0000x reference)
"""Trainium2 Bass kernel for nn_Encoder: B=1M samples through
concat(x,c) -> per-j Linear(5,3)+ReLU -> Linear(51,32)+ReLU ->
Linear(32,16)+ReLU -> {Linear(16,3) mu, Linear(16,3) log_var}.

Strategy: pure data parallel over 8 NeuronCores. The host pre-packs each
core's shard FEATURE-MAJOR in bf16 (xct [85, 131072]): feature rows on
SBUF partitions, samples along the free dim, so matmul rhs tiles come
straight from the DMA with no on-chip transpose at all. Per 512-sample
tile the pipeline is 4 block-diagonal bf16 matmuls with fp32 PSUM
accumulation and fused bias+ReLU (DVE/ACT):
  L1: K=85  -> M=64 (51 real), two tiles packed into one [128,512] psum
  L2: K=128 -> M=64, 2-tile pairs packed               -> relu [128,512]
  L3: K=128 -> M=64 (4 tiles)                          -> relu [128,512]
  heads: K=128 -> M=48 (8 tiles, mu+lv interleaved), fp32 bias-add
The head output [48, 512] is stored as-is (device layout); the host
un-permutes and splits mu/lv. bf16 input rounding gives ~4e-3 max
scale-relative error vs the fp32 reference (numpy-simulated).
"""
import numpy as np
import ml_dtypes

import concourse.bass as bass
import concourse.mybir as mybir
import concourse.tile as tile
from concourse.bass_utils import run_bass_kernel_spmd

AF = mybir.ActivationFunctionType
ALU = mybir.AluOpType
F32 = mybir.dt.float32
BF16 = mybir.dt.bfloat16
BF16_NP = ml_dtypes.bfloat16

N_CORES = 8
B_FULL = 1_000_000
PER_CORE = B_FULL // N_CORES      # 125000
BLK = 8192                        # samples per load block
NBLK = 16                         # load blocks per core
NTOT = BLK * NBLK                 # 131072 padded samples per core
TILES = BLK // 512                # 16 tiles of 512 samples per block
NF = 85                           # feature rows (51 from x, 34 from c)

_OFF_W1, _OFF_W2, _OFF_W3, _OFF_WH = 0, 64, 128, 192
_WCOLS = 240


def _host_packs(W1, b1, W2, b2, W3, b3, Wmu, bmu, Wlv, blv):
    """Block-diagonal bf16 weights [128, 240] + fp32 bias pack [128, 4]."""
    W1blk = np.zeros((128, 64), np.float32)
    for j in range(17):
        for o in range(3):
            for k in range(3):
                W1blk[3 * j + k, 3 * j + o] = W1[o, k]
            for k in range(2):
                W1blk[51 + 2 * j + k, 3 * j + o] = W1[o, 3 + k]
    W2blk = np.zeros((128, 64), np.float32)
    W2blk[0:51, 0:32] = W2.T
    W2blk[64:115, 32:64] = W2.T
    W3blk = np.zeros((128, 64), np.float32)
    for t in range(4):
        W3blk[32 * t:32 * t + 32, 16 * t:16 * t + 16] = W3.T
    Wh = np.concatenate([Wmu, Wlv], axis=0)          # [6, 16]
    Whblk = np.zeros((128, 48), np.float32)
    for t in range(8):
        Whblk[16 * t:16 * t + 16, 6 * t:6 * t + 6] = Wh.T
    wpack = np.concatenate([W1blk, W2blk, W3blk, Whblk],
                           axis=1).astype(BF16_NP)   # [128, 240]

    b1v = np.zeros((128,), np.float32)
    for j in range(17):
        for o in range(3):
            b1v[3 * j + o] = b1[o]
            b1v[64 + 3 * j + o] = b1[o]
    b2v = np.tile(b2, 4).astype(np.float32)
    b3v = np.tile(b3, 8).astype(np.float32)
    bh = np.concatenate([bmu, blv])
    bhv = np.zeros((128,), np.float32)
    bhv[0:48] = np.tile(bh, 8)
    bpack = np.stack([b1v, b2v, b3v, bhv], axis=1)   # [128, 4]
    return wpack, bpack


def _prep_core(x_flat, c_flat):
    """[n, 51] + [n, 34] fp32 -> feature-major bf16 [85, NTOT]."""
    n = x_flat.shape[0]
    xct = np.zeros((NF, NTOT), BF16_NP)
    xct[0:51, :n] = x_flat.T
    xct[51:85, :n] = c_flat.T
    return xct


def _unpack_out(out_dev):
    """Device layout [48, NBLK*1024] fp32 -> (mu, lv) [NTOT, 3]."""
    arr = out_dev.reshape(8, 6, NBLK, 2, 512)        # [t', o, b, g, c]
    arr = np.transpose(arr, (2, 3, 0, 4, 1))         # [b, g, t', c, o]
    arr = arr.reshape(NTOT, 6)
    return arr[:, 0:3], arr[:, 3:6]


def build_kernel(nblk=NBLK):
    ntot = BLK * nblk
    nc = bass.Bass("TRN2")
    xcd = nc.dram_tensor("xct", [NF, ntot], BF16, kind="ExternalInput")
    wd = nc.dram_tensor("wpack", [128, _WCOLS], BF16, kind="ExternalInput")
    bd = nc.dram_tensor("bpack", [128, 4], F32, kind="ExternalInput")
    od = nc.dram_tensor("out_dev", [48, nblk * 1024], F32,
                        kind="ExternalOutput")

    with tile.TileContext(nc) as tc:
        with tc.tile_pool(name="const", bufs=1) as constp, \
             tc.tile_pool(name="xc", bufs=3) as xcp, \
             tc.tile_pool(name="h1", bufs=2) as h1p, \
             tc.tile_pool(name="h2", bufs=2) as h2p, \
             tc.tile_pool(name="h3", bufs=2) as h3p, \
             tc.tile_pool(name="h4", bufs=2) as h4p, \
             tc.tile_pool(name="ps1", bufs=2, space="PSUM") as ps1p, \
             tc.tile_pool(name="ps2", bufs=2, space="PSUM") as ps2p, \
             tc.tile_pool(name="ps3", bufs=2, space="PSUM") as ps3p, \
             tc.tile_pool(name="ps4", bufs=2, space="PSUM") as ps4p:

            wt = constp.tile([128, _WCOLS], BF16)
            bt = constp.tile([128, 4], F32)
            nc.sync.dma_start(out=wt, in_=wd[:, :])
            nc.sync.dma_start(out=bt, in_=bd[:, :])
            w1 = wt[0:NF, _OFF_W1:_OFF_W1 + 64]
            w2 = wt[:, _OFF_W2:_OFF_W2 + 64]
            w3 = wt[:, _OFF_W3:_OFF_W3 + 64]
            wh = wt[:, _OFF_WH:_OFF_WH + 48]
            b1v = bt[:, 0:1]
            b2v = bt[:, 1:2]
            b3v = bt[:, 2:3]
            bhv = bt[0:48, 3:4]

            psum1 = psum2 = psum3 = None
            for b in range(nblk):
                xcb = xcp.tile([NF, BLK], BF16)
                nc.sync.dma_start(out=xcb, in_=xcd[:, b * BLK:(b + 1) * BLK])
                h4buf = h4p.tile([48, 1024], F32)

                for t in range(TILES):
                    rhs0 = xcb[:, 512 * t:512 * (t + 1)]
                    half = (t % 2) * 64
                    if t % 2 == 0:
                        psum1 = ps1p.tile([128, 512], F32)
                    nc.tensor.matmul(psum1[half:half + 64, :], w1, rhs0,
                                     start=True, stop=True)
                    if t % 2 != 1:
                        continue
                    h1 = h1p.tile([128, 512], BF16)
                    nc.vector.tensor_scalar(
                        out=h1, in0=psum1, scalar1=b1v, scalar2=0.0,
                        op0=ALU.add, op1=ALU.max)

                    u = t // 2
                    half = (u % 2) * 64
                    if u % 2 == 0:
                        psum2 = ps2p.tile([128, 512], F32)
                    nc.tensor.matmul(psum2[half:half + 64, :], w2, h1,
                                     start=True, stop=True)
                    if u % 2 != 1:
                        continue
                    h2 = h2p.tile([128, 512], BF16)
                    nc.scalar.activation(h2, psum2, AF.Relu, bias=b2v)

                    v = t // 4
                    half = (v % 2) * 64
                    if v % 2 == 0:
                        psum3 = ps3p.tile([128, 512], F32)
                    nc.tensor.matmul(psum3[half:half + 64, :], w3, h2,
                                     start=True, stop=True)
                    if v % 2 != 1:
                        continue
                    h3 = h3p.tile([128, 512], BF16)
                    nc.scalar.activation(h3, psum3, AF.Relu, bias=b3v)

                    g = t // 8
                    psum4 = ps4p.tile([48, 512], F32)
                    nc.tensor.matmul(psum4, wh, h3, start=True, stop=True)
                    nc.scalar.activation(h4buf[:, 512 * g:512 * (g + 1)],
                                         psum4, AF.Identity, bias=bhv)

                nc.sync.dma_start(out=od[:, b * 1024:(b + 1) * 1024],
                                  in_=h4buf)

    from tile_patch import split_excess_waits
    split_excess_waits(nc)
    return nc


_NC_CACHE = {}


def _get_nc(nblk=NBLK):
    if nblk not in _NC_CACHE:
        _NC_CACHE[nblk] = build_kernel(nblk)
    return _NC_CACHE[nblk]


def kernel(x, c, W1, b1, W2, b2, W3, b3, Wmu, bmu, Wlv, blv, _trace=False):
    x = np.asarray(x, np.float32).reshape(B_FULL, 51)
    c = np.asarray(c, np.float32).reshape(B_FULL, 34)
    wpack, bpack = _host_packs(
        np.asarray(W1, np.float32), np.asarray(b1, np.float32),
        np.asarray(W2, np.float32), np.asarray(b2, np.float32),
        np.asarray(W3, np.float32), np.asarray(b3, np.float32),
        np.asarray(Wmu, np.float32), np.asarray(bmu, np.float32),
        np.asarray(Wlv, np.float32), np.asarray(blv, np.float32))

    in_maps = []
    for core in range(N_CORES):
        sl = slice(core * PER_CORE, (core + 1) * PER_CORE)
        in_maps.append({"xct": _prep_core(x[sl], c[sl]),
                        "wpack": wpack, "bpack": bpack})

    nc = _get_nc()
    res = run_bass_kernel_spmd(nc, in_maps, core_ids=list(range(N_CORES)),
                               trace=_trace)
    mus, lvs = [], []
    for i in range(N_CORES):
        mu_i, lv_i = _unpack_out(res.results[i]["out_dev"])
        mus.append(mu_i[:PER_CORE])
        lvs.append(lv_i[:PER_CORE])
    out = (np.concatenate(mus), np.concatenate(lvs))
    if _trace:
        return out, res
    return out


# revision 17
# speedup vs baseline: 770.2708x; 770.2708x over previous
"""Trainium2 Bass kernel for nn_Encoder: B=1M samples through
concat(x,c) -> per-j Linear(5,3)+ReLU -> Linear(51,32)+ReLU ->
Linear(32,16)+ReLU -> {Linear(16,3) mu, Linear(16,3) log_var}.

Strategy: pure data parallel over 8 NeuronCores. The host pre-packs each
core's shard FEATURE-MAJOR in bf16 (xct [85, 131072]): feature rows on
SBUF partitions, samples along the free dim, so matmul rhs tiles come
straight from the DMA with no on-chip transpose at all. Per 512-sample
tile the pipeline is 4 block-diagonal bf16 matmuls with fp32 PSUM
accumulation and fused bias+ReLU (DVE/ACT):
  L1: K=85  -> M=64 (51 real), two tiles packed into one [128,512] psum
  L2: K=128 -> M=64, 2-tile pairs packed               -> relu [128,512]
  L3: K=128 -> M=64 (4 tiles)                          -> relu [128,512]
  heads: K=128 -> M=48 (8 tiles, mu+lv interleaved), fp32 bias-add
The head output [48, 512] is stored as-is (device layout); the host
un-permutes and splits mu/lv. bf16 input rounding gives ~4e-3 max
scale-relative error vs the fp32 reference (numpy-simulated).
"""
import numpy as np
import ml_dtypes

import concourse.bass as bass
import concourse.mybir as mybir
import concourse.tile as tile
from concourse.bass_utils import run_bass_kernel_spmd

AF = mybir.ActivationFunctionType
ALU = mybir.AluOpType
F32 = mybir.dt.float32
BF16 = mybir.dt.bfloat16
BF16_NP = ml_dtypes.bfloat16

N_CORES = 8
B_FULL = 1_000_000
PER_CORE = B_FULL // N_CORES      # 125000
BLK = 8192                        # samples per load block
NBLK = 16                         # load blocks per core
NTOT = BLK * NBLK                 # 131072 padded samples per core
TILES = BLK // 512                # 16 tiles of 512 samples per block
NF = 85                           # feature rows (51 from x, 34 from c)

_OFF_W1, _OFF_W2, _OFF_W3, _OFF_WH = 0, 64, 128, 192
_WCOLS = 240


# --- walrus sync-wait-limit workaround (inlined) -------------------------
# The ISA sync slots allow only one wait per regular instruction, but
# Tile's wait assigner can attach several (tail drain, multi-dep
# consumers, self-loading matmuls). Post-pass: move excess waits onto
# freshly inserted same-engine NoOps placed immediately before the needy
# instruction - identical sync semantics, one wait per instruction.
_ws_ctr = [0]


def _split_excess_waits(nc, max_waits=1):
    for fn in nc.m.functions:
        for bb in fn.blocks:
            insts = bb.instructions
            i = 0
            while i < len(insts):
                inst = insts[i]
                si = inst.sync_info
                if si is None or si.on_wait is None or \
                        len(si.on_wait) <= max_waits:
                    i += 1
                    continue
                waits = list(si.on_wait)
                keep = waits[-max_waits:]
                excess = waits[:-max_waits]
                new_nops = []
                for w in excess:
                    _ws_ctr[0] += 1
                    nop = mybir.InstNoOp(
                        name=f"I-waitsplit-{_ws_ctr[0]}",
                        sync_info=mybir.SyncInfo(on_wait=[w], on_update=[]),
                        bass_nofuse=True,
                        engine=inst.engine,
                    )
                    new_nops.append(nop)
                inst.sync_info = mybir.SyncInfo(
                    on_wait=keep, on_update=list(si.on_update or []))
                for j, nop in enumerate(new_nops):
                    insts.insert(i + j, nop)
                i += len(new_nops) + 1



def _host_packs(W1, b1, W2, b2, W3, b3, Wmu, bmu, Wlv, blv):
    """Block-diagonal bf16 weights [128, 240] + fp32 bias pack [128, 4]."""
    W1blk = np.zeros((128, 64), np.float32)
    for j in range(17):
        for o in range(3):
            for k in range(3):
                W1blk[3 * j + k, 3 * j + o] = W1[o, k]
            for k in range(2):
                W1blk[51 + 2 * j + k, 3 * j + o] = W1[o, 3 + k]
    W2blk = np.zeros((128, 64), np.float32)
    W2blk[0:51, 0:32] = W2.T
    W2blk[64:115, 32:64] = W2.T
    W3blk = np.zeros((128, 64), np.float32)
    for t in range(4):
        W3blk[32 * t:32 * t + 32, 16 * t:16 * t + 16] = W3.T
    Wh = np.concatenate([Wmu, Wlv], axis=0)          # [6, 16]
    Whblk = np.zeros((128, 48), np.float32)
    for t in range(8):
        Whblk[16 * t:16 * t + 16, 6 * t:6 * t + 6] = Wh.T
    wpack = np.concatenate([W1blk, W2blk, W3blk, Whblk],
                           axis=1).astype(BF16_NP)   # [128, 240]

    b1v = np.zeros((128,), np.float32)
    for j in range(17):
        for o in range(3):
            b1v[3 * j + o] = b1[o]
            b1v[64 + 3 * j + o] = b1[o]
    b2v = np.tile(b2, 4).astype(np.float32)
    b3v = np.tile(b3, 8).astype(np.float32)
    bh = np.concatenate([bmu, blv])
    bhv = np.zeros((128,), np.float32)
    bhv[0:48] = np.tile(bh, 8)
    bpack = np.stack([b1v, b2v, b3v, bhv], axis=1)   # [128, 4]
    return wpack, bpack


def _prep_core(x_flat, c_flat):
    """[n, 51] + [n, 34] fp32 -> feature-major bf16 [128, NTOT].

    Rows 85-127 stay zero: 128-partition DMA destinations run ~2.5x
    faster than 85-row ones (measured 383 vs 150 GB/s), which more than
    pays for the 50% pad bytes; the L1 weights for rows 85-127 are 0."""
    n = x_flat.shape[0]
    xct = np.zeros((128, NTOT), BF16_NP)
    xct[0:51, :n] = x_flat.T
    xct[51:85, :n] = c_flat.T
    return xct


def _unpack_out(out_dev):
    """Device layout [48, NBLK*1024] fp32 -> (mu, lv) [NTOT, 3]."""
    arr = out_dev.reshape(8, 6, NBLK, 2, 512)        # [t', o, b, g, c]
    arr = np.transpose(arr, (2, 3, 0, 4, 1))         # [b, g, t', c, o]
    arr = arr.reshape(NTOT, 6)
    return arr[:, 0:3], arr[:, 3:6]


def build_kernel(nblk=NBLK, repeat=1, mode='full'):
    ntot = BLK * nblk
    nc = bass.Bass("TRN2")
    xcd = nc.dram_tensor("xct", [128, ntot], BF16, kind="ExternalInput")
    wd = nc.dram_tensor("wpack", [128, _WCOLS], BF16, kind="ExternalInput")
    bd = nc.dram_tensor("bpack", [128, 4], F32, kind="ExternalInput")
    od = nc.dram_tensor("out_dev", [48, nblk * 1024], F32,
                        kind="ExternalOutput")

    with tile.TileContext(nc) as tc:
        with tc.tile_pool(name="const", bufs=1) as constp, \
             tc.tile_pool(name="xc", bufs=3) as xcp, \
             tc.tile_pool(name="h1", bufs=2) as h1p, \
             tc.tile_pool(name="h2", bufs=2) as h2p, \
             tc.tile_pool(name="h3", bufs=2) as h3p, \
             tc.tile_pool(name="h4", bufs=2) as h4p, \
             tc.tile_pool(name="ps1", bufs=2, space="PSUM") as ps1p, \
             tc.tile_pool(name="ps2", bufs=2, space="PSUM") as ps2p, \
             tc.tile_pool(name="ps3", bufs=2, space="PSUM") as ps3p, \
             tc.tile_pool(name="ps4", bufs=2, space="PSUM") as ps4p:

            wt = constp.tile([128, _WCOLS], BF16)
            bt = constp.tile([128, 4], F32)
            nc.sync.dma_start(out=wt, in_=wd[:, :])
            nc.sync.dma_start(out=bt, in_=bd[:, :])
            w1 = wt[:, _OFF_W1:_OFF_W1 + 64]
            w2 = wt[:, _OFF_W2:_OFF_W2 + 64]
            w3 = wt[:, _OFF_W3:_OFF_W3 + 64]
            wh = wt[:, _OFF_WH:_OFF_WH + 48]
            b1v = bt[:, 0:1]
            b2v = bt[:, 1:2]
            b3v = bt[:, 2:3]
            bhv = bt[0:48, 3:4]

            psum1 = psum2 = psum3 = None
            for b in [bb for _ in range(repeat) for bb in range(nblk)]:
                xcb = xcp.tile([128, BLK], BF16)
                if mode != 'nodma':
                    nc.sync.dma_start(out=xcb,
                                      in_=xcd[:, b * BLK:(b + 1) * BLK])
                else:
                    nc.vector.memset(xcb[:, 0:2].bitcast(mybir.dt.uint32), 0)
                h4buf = h4p.tile([48, 1024], F32)
                if mode == 'peonly':
                    nc.vector.memset(h4buf[:, 0:2], 0.0)

                if mode == 'dmaonly':
                    nc.vector.tensor_copy(h4buf[:, 0:512], xcb[0:48, 0:512])
                    nc.sync.dma_start(out=od[:, b * 1024:(b + 1) * 1024],
                                      in_=h4buf)
                    continue
                if mode == 'inonly':
                    nc.vector.tensor_copy(h4buf[:, 0:2], xcb[0:48, 0:2])
                    continue
                if mode == 'in128':
                    flat = xcd.rearrange("f (a c) -> (f a) c", c=2048)
                    nrows = NF * (ntot // 2048)
                    for i3 in range(3):
                        st = (128 * (3 * b + i3)) % (nrows - 128)
                        xcb128 = xcp.tile([128, 2048], BF16, tag="xcb128")
                        nc.sync.dma_start(out=xcb128,
                                          in_=flat[st:st + 128, :])
                        nc.vector.tensor_copy(h4buf[:, 2 * i3:2 * i3 + 2],
                                              xcb128[0:48, 0:2])
                    continue
                if mode == 'inbig':
                    if b % 2 == 0:
                        xcb2 = xcp.tile([NF, 2 * BLK], BF16, tag="xcb2")
                        nc.sync.dma_start(
                            out=xcb2,
                            in_=xcd[:, b * BLK:(b + 2) * BLK])
                        nc.vector.tensor_copy(h4buf[:, 0:2], xcb2[0:48, 0:2])
                    continue
                for t in range(TILES):
                    rhs0 = xcb[:, 512 * t:512 * (t + 1)]
                    half = (t % 2) * 64
                    if t % 2 == 0:
                        psum1 = ps1p.tile([128, 512], F32)
                    nc.tensor.matmul(psum1[half:half + 64, :], w1, rhs0,
                                     start=True, stop=True)
                    if t % 2 != 1:
                        continue
                    h1 = h1p.tile([128, 512], BF16)
                    if mode == 'peonly':
                        nc.vector.memset(h1[:, 0:2].bitcast(mybir.dt.uint32), 0)
                    else:
                        nc.vector.tensor_scalar(
                            out=h1, in0=psum1, scalar1=b1v, scalar2=0.0,
                            op0=ALU.add, op1=ALU.max)

                    u = t // 2
                    half = (u % 2) * 64
                    if u % 2 == 0:
                        psum2 = ps2p.tile([128, 512], F32)
                    nc.tensor.matmul(psum2[half:half + 64, :], w2, h1,
                                     start=True, stop=True)
                    if u % 2 != 1:
                        continue
                    h2 = h2p.tile([128, 512], BF16)
                    if mode == 'peonly':
                        nc.vector.memset(h2[:, 0:2].bitcast(mybir.dt.uint32), 0)
                    else:
                        nc.scalar.activation(h2, psum2, AF.Relu, bias=b2v)

                    v = t // 4
                    half = (v % 2) * 64
                    if v % 2 == 0:
                        psum3 = ps3p.tile([128, 512], F32)
                    nc.tensor.matmul(psum3[half:half + 64, :], w3, h2,
                                     start=True, stop=True)
                    if v % 2 != 1:
                        continue
                    h3 = h3p.tile([128, 512], BF16)
                    if mode == 'peonly':
                        nc.vector.memset(h3[:, 0:2].bitcast(mybir.dt.uint32), 0)
                    else:
                        nc.scalar.activation(h3, psum3, AF.Relu, bias=b3v)

                    g = t // 8
                    psum4 = ps4p.tile([48, 512], F32)
                    nc.tensor.matmul(psum4, wh, h3, start=True, stop=True)
                    if mode != 'peonly':
                        nc.scalar.activation(h4buf[:, 512 * g:512 * (g + 1)],
                                             psum4, AF.Identity, bias=bhv)

                nc.sync.dma_start(out=od[:, b * 1024:(b + 1) * 1024],
                                  in_=h4buf)

    _split_excess_waits(nc)
    return nc


_NC_CACHE = {}


def _get_nc(nblk=NBLK, repeat=1, mode='full'):
    key = (nblk, repeat, mode)
    if key not in _NC_CACHE:
        _NC_CACHE[key] = build_kernel(nblk, repeat, mode)
    return _NC_CACHE[key]


def kernel(x, c, W1, b1, W2, b2, W3, b3, Wmu, bmu, Wlv, blv, _trace=False):
    x = np.asarray(x, np.float32).reshape(B_FULL, 51)
    c = np.asarray(c, np.float32).reshape(B_FULL, 34)
    wpack, bpack = _host_packs(
        np.asarray(W1, np.float32), np.asarray(b1, np.float32),
        np.asarray(W2, np.float32), np.asarray(b2, np.float32),
        np.asarray(W3, np.float32), np.asarray(b3, np.float32),
        np.asarray(Wmu, np.float32), np.asarray(bmu, np.float32),
        np.asarray(Wlv, np.float32), np.asarray(blv, np.float32))

    in_maps = []
    for core in range(N_CORES):
        sl = slice(core * PER_CORE, (core + 1) * PER_CORE)
        in_maps.append({"xct": _prep_core(x[sl], c[sl]),
                        "wpack": wpack, "bpack": bpack})

    nc = _get_nc()
    res = run_bass_kernel_spmd(nc, in_maps, core_ids=list(range(N_CORES)),
                               trace=_trace)
    mus, lvs = [], []
    for i in range(N_CORES):
        mu_i, lv_i = _unpack_out(res.results[i]["out_dev"])
        mus.append(mu_i[:PER_CORE])
        lvs.append(lv_i[:PER_CORE])
    out = (np.concatenate(mus), np.concatenate(lvs))
    if _trace:
        return out, res
    return out
